# revision 10
# baseline (speedup 1.0000x reference)
"""Trainium2 Bass kernel for MultiHeadSelfAttention with relative position bias.

Sharding: 8 cores = 2 batches x 4 head-groups (4 heads each).
Each core computes LN -> QKV (its heads) -> scores+softmax+AV -> partial
out_proj; host sums the 4 partials per batch (the all-reduce) and adds b_out.

Tricks:
1. rel_bias[i-j] is exactly rank-64 in (i, j) (sinusoid angle-addition), so
   it folds into the scores matmul as 64 extra contraction rows:
       scores+bias = [kT; Kb]^T @ [qT_scaled; Qb]   (K = 128, full PE array)
2. LayerNorm never materializes xn: projections run on unnormalized xT
   (host-transposed), and LN folds in algebraically:
       (xn @ W)^T[c,i] = rstd_i * (P[c,i] - mu_i * colsum(W)_c) + b_c
   The rank-1/2 corrections are tiny K<=2 matmuls accumulated into the same
   PSUM group; the rstd_i scaling rides the PSUM->SBUF evacuation.
3. Softmax denominators come free via an all-ones column appended to V
   (row 64 of the AV output = sum_j exp); normalization is deferred across
   the (linear) AV matmul into the OT evacuation.
"""

import functools
import math
import sys

import numpy as np

for _p in ("/opt/trn_rl_repo", "/root/.axon_site/_ro/trn_rl_repo"):
    if _p not in sys.path:
        sys.path.insert(0, _p)

B, T, D, H, HD = 2, 2048, 1024, 16, 64
NCORES = 8
HPC = 4  # heads per core
NT = T // 128  # 16 row tiles
NI = T // 512  # 4 i-chunks
F32R = True  # float32r (full-rate) matmuls; False = exact fp32 at 1/4 rate


@functools.lru_cache(maxsize=1)
def _build_nc():
    import concourse.mybir as mybir
    import concourse.tile as tile
    from concourse import bacc

    dt = mybir.dt
    f32 = dt.float32
    f32r = dt.float32r if F32R else f32
    AF = mybir.ActivationFunctionType

    nc = bacc.Bacc("TRN2", target_bir_lowering=False, debug=False,
                   num_devices=NCORES)
    x = nc.declare_dram_parameter("x", [T, D], f32, isOutput=False)
    xt = nc.declare_dram_parameter("xt", [D, T], f32r, isOutput=False)
    wq = nc.declare_dram_parameter("wq", [D, 256], f32r, isOutput=False)
    wk = nc.declare_dram_parameter("wk", [D, 256], f32r, isOutput=False)
    wv = nc.declare_dram_parameter("wv", [D, 260], f32r, isOutput=False)
    qcor = nc.declare_dram_parameter("qcor", [2, 256], f32, isOutput=False)
    kcor = nc.declare_dram_parameter("kcor", [1, 256], f32, isOutput=False)
    vcor = nc.declare_dram_parameter("vcor", [2, 260], f32, isOutput=False)
    qb = nc.declare_dram_parameter("qb", [HPC, 64, T], f32r, isOutput=False)
    kb = nc.declare_dram_parameter("kb", [64, T], f32r, isOutput=False)
    wo = nc.declare_dram_parameter("wo", [256, D], f32r, isOutput=False)
    y = nc.declare_dram_parameter("y", [T, D], f32, isOutput=True)

    with tile.TileContext(nc) as tc:
        with tc.tile_pool(name="persist", bufs=1) as pp, \
             tc.tile_pool(name="big_ps", bufs=2, space="PSUM") as bigp, \
             tc.tile_pool(name="ot_ps", bufs=2, space="PSUM") as otpp:
            zero_col = pp.tile([128, 1], f32, tag="zero")
            nc.vector.memset(zero_col, 0.0)
            qhat = [pp.tile([128, T], f32r, tag=f"qhat{h}", name=f"qhat{h}")
                    for h in range(HPC)]
            khat = [pp.tile([128, T], f32r, tag=f"khat{h}", name=f"khat{h}")
                    for h in range(HPC)]
            v_sb = [pp.tile([128, 260], f32r, tag=f"v{jt}", name=f"v{jt}")
                    for jt in range(NT)]
            ot_pair = [pp.tile([128, T], f32r, tag=f"ot{p}", name=f"otp{p}")
                       for p in range(2)]
            wq_sb = pp.tile([128, 8, 256], f32r, tag="wq")
            wk_sb = pp.tile([128, 8, 256], f32r, tag="wk")
            wv_sb = pp.tile([128, 8, 260], f32r, tag="wv")

            # ------------- Phase A+B interleaved per i-group -------------
            with tc.tile_pool(name="ab_sb", bufs=1) as abp, \
                 tc.tile_pool(name="ln", bufs=3) as lnp, \
                 tc.tile_pool(name="stats", bufs=3) as stp, \
                 tc.tile_pool(name="mvp", bufs=1) as mvp, \
                 tc.tile_pool(name="rows", bufs=2) as rwp, \
                 tc.tile_pool(name="xts", bufs=2) as xtp, \
                 tc.tile_pool(name="drb", bufs=1, space="DRAM") as drp:
                eps_t = abp.tile([128, 1], f32, tag="eps")
                nc.vector.memset(eps_t, 1e-5)
                qcor_sb = abp.tile([2, 256], f32, tag="qcor")
                nc.sync.dma_start(out=qcor_sb, in_=qcor[:, :])
                kcor_sb = abp.tile([1, 256], f32, tag="kcor")
                nc.sync.dma_start(out=kcor_sb, in_=kcor[:, :])
                vcor_sb = abp.tile([2, 260], f32, tag="vcor")
                nc.sync.dma_start(out=vcor_sb, in_=vcor[:, :])
                # DRAM bounce rows: [mu; sq] and rstd
                mus_d = drp.tile([2, T], f32, tag="musd")
                rst_d = drp.tile([1, T], f32, tag="rstd")

                rstd_cols = []
                for it in range(NT):
                    x_t = lnp.tile([128, D], f32, tag="x", name=f"x{it}")
                    nc.sync.dma_start(
                        out=x_t, in_=x[it * 128:(it + 1) * 128, :])
                    stats = stp.tile([128, 2, 6], f32, tag="st")
                    for sg in range(2):
                        nc.vector.bn_stats(
                            out=stats[:, sg, :],
                            in_=x_t[:, sg * 512:(sg + 1) * 512])
                    mv = mvp.tile([128, 2], f32, tag=f"mv{it}",
                                  name=f"mv{it}")
                    nc.vector.bn_aggr(out=mv, in_=stats)
                    # mv[:,1] = sqrt(var+eps); bounce mu and sq to DRAM rows
                    nc.scalar.activation(
                        out=mv[:, 1:2], in_=mv[:, 1:2], func=AF.Sqrt,
                        bias=eps_t, scale=1.0)
                    tsl = slice(it * 128, (it + 1) * 128)
                    nc.sync.dma_start(out=mus_d[0:1, tsl], in_=mv[:, 0:1])
                    nc.sync.dma_start(out=mus_d[1:2, tsl], in_=mv[:, 1:2])
                    nc.vector.reciprocal_approx_fast(
                        out=mv[:, 1:2], in_=mv[:, 1:2])
                    nc.sync.dma_start(out=rst_d[0:1, tsl], in_=mv[:, 1:2])
                    rstd_cols.append(mv)

                for ig in range(NI):
                    isl = slice(ig * 512, (ig + 1) * 512)
                    # per-ig stat rows + broadcast
                    mus_r = rwp.tile([2, 512], f32, tag="musr",
                                     name=f"musr{ig}")
                    nc.sync.dma_start(out=mus_r, in_=mus_d[:, isl])
                    rst_r = rwp.tile([1, 512], f32, tag="rstr",
                                     name=f"rstr{ig}")
                    nc.sync.dma_start(out=rst_r, in_=rst_d[:, isl])
                    rst_bc = rwp.tile([128, 512], f32, tag="rstb",
                                      name=f"rstb{ig}")
                    nc.gpsimd.partition_broadcast(rst_bc, rst_r)

                    xts = []
                    for kc in range(8):
                        xv = xtp.tile([128, 512], f32r, tag=f"x{kc}",
                                      name=f"xt{kc}_{ig}")
                        nc.sync.dma_start(
                            out=xv,
                            in_=xt[kc * 128:(kc + 1) * 128, isl])
                        xts.append(xv)

                    if ig == 0:
                        nc.sync.dma_start(
                            out=wq_sb,
                            in_=wq[:, :].rearrange("(kc p) c -> p kc c", p=128))
                        nc.sync.dma_start(
                            out=wk_sb,
                            in_=wk[:, :].rearrange("(kc p) c -> p kc c", p=128))
                        nc.sync.dma_start(
                            out=wv_sb,
                            in_=wv[:, :].rearrange("(kc p) c -> p kc c", p=128))

                    # q/k projections for this i-chunk (both cc in one tile)
                    for wt, dest, cor in ((wq_sb, qhat, qcor_sb),
                                          (wk_sb, khat, kcor_sb)):
                        ps = bigp.tile([128, 1024], f32, tag="big",
                                       name=f"qk{ig}")
                        for cc in range(2):
                            psl = ps[:, cc * 512:(cc + 1) * 512]
                            csl = slice(cc * 128, (cc + 1) * 128)
                            for kc in range(8):
                                nc.tensor.matmul(
                                    psl, lhsT=wt[:, kc, csl], rhs=xts[kc],
                                    start=(kc == 0), stop=False)
                            kr = cor.shape[0]
                            nc.tensor.matmul(
                                psl, lhsT=cor[:, csl],
                                rhs=mus_r[0:kr, :], start=False, stop=True)
                            nc.vector.tensor_mul(
                                out=dest[2 * cc][0:64, isl],
                                in0=psl[0:64, :], in1=rst_bc[0:64, :])
                            nc.vector.tensor_mul(
                                out=dest[2 * cc + 1][0:64, isl],
                                in0=psl[64:128, :], in1=rst_bc[64:128, :])

                    # v projections for the 4 j-tiles of this group
                    for jp in range(2):
                        ps = bigp.tile([128, 1024], f32, tag="big",
                                       name=f"vv{ig}_{jp}")
                        for half in range(2):
                            r = jp * 2 + half
                            jt = ig * 4 + r
                            psl = ps[:, half * 512:half * 512 + 260]
                            for kc in range(8):
                                nc.tensor.matmul(
                                    psl,
                                    lhsT=xts[kc][:, r * 128:(r + 1) * 128],
                                    rhs=wv_sb[:, kc, :],
                                    start=(kc == 0), stop=False)
                            nc.tensor.matmul(
                                psl,
                                lhsT=mus_r[:, r * 128:(r + 1) * 128],
                                rhs=vcor_sb, start=False, stop=True)
                            nc.vector.tensor_scalar_mul(
                                out=v_sb[jt], in0=psl,
                                scalar1=rstd_cols[jt][:, 1:2])

                # bias factor tables (needed first by phase C)
                for h in range(HPC):
                    nc.sync.dma_start(out=qhat[h][64:128, :], in_=qb[h])
                    nc.sync.dma_start(out=khat[h][64:128, :], in_=kb[:, :])

            # ---------------- Phase C: attention per head ----------------
            with tc.tile_pool(name="pt", bufs=4) as ptp, \
                 tc.tile_pool(name="rr", bufs=4) as rrp, \
                 tc.tile_pool(name="rb", bufs=4) as rbp:
                for h in range(HPC):
                    vsl = slice(h * 65, (h + 1) * 65)
                    for ic2 in range(2):
                        ps_o = [otpp.tile([128, 512], f32, tag=f"ot{_o}",
                                          name=f"ot{_o}_{h}_{ic2}")
                                for _o in range(2)]
                        for jt in range(NT):
                            jsl = slice(jt * 128, (jt + 1) * 128)
                            st = bigp.tile([128, 1024], f32, tag="big",
                                           name=f"sc{h}_{ic2}_{jt}")
                            pt = ptp.tile([128, 1024], f32r, tag="pt")
                            for half in range(2):
                                i0 = ic2 * 1024 + half * 512
                                nc.tensor.matmul(
                                    st[:, half * 512:(half + 1) * 512],
                                    lhsT=khat[h][:, jsl],
                                    rhs=qhat[h][:, i0:i0 + 512],
                                    start=True, stop=True)
                            nc.scalar.activation(
                                out=pt, in_=st, func=AF.Exp,
                                bias=zero_col, scale=1.0)
                            for half in range(2):
                                nc.tensor.matmul(
                                    ps_o[half][0:65, :],
                                    lhsT=v_sb[jt][:, vsl],
                                    rhs=pt[:, half * 512:(half + 1) * 512],
                                    start=(jt == 0), stop=(jt == NT - 1))
                        hp, pair = h % 2, h // 2
                        for half in range(2):
                            i0 = ic2 * 1024 + half * 512
                            rr = rrp.tile([1, 512], f32, tag="rr")
                            # approx-recip mis-reads PSUM; bounce via SBUF
                            nc.vector.tensor_copy(
                                out=rr, in_=ps_o[half][64:65, :])
                            nc.vector.reciprocal_approx_fast(out=rr, in_=rr)
                            rb = rbp.tile([64, 512], f32, tag="rb")
                            nc.gpsimd.partition_broadcast(rb, rr)
                            nc.vector.tensor_mul(
                                out=ot_pair[pair][hp * 64:(hp + 1) * 64,
                                                  i0:i0 + 512],
                                in0=ps_o[half][0:64, :], in1=rb)

            # ---------------- Phase D: out projection ----------------
            with tc.tile_pool(name="yp", bufs=3) as yp, \
                 tc.tile_pool(name="wop", bufs=1) as wop:
                wo_sb = wop.tile([128, 2, D], f32r, tag="wo")
                nc.sync.dma_start(
                    out=wo_sb,
                    in_=wo[:, :].rearrange("(pc p) n -> p pc n", p=128))
                for it in range(NT):
                    tsl = slice(it * 128, (it + 1) * 128)
                    psy = bigp.tile([128, D], f32, tag="big", name=f"y{it}")
                    for nh in range(2):
                        for p in range(2):
                            nc.tensor.matmul(
                                psy[:, nh * 512:(nh + 1) * 512],
                                lhsT=ot_pair[p][:, tsl],
                                rhs=wo_sb[:, p, nh * 512:(nh + 1) * 512],
                                start=(p == 0), stop=(p == 1))
                    y_t = yp.tile([128, D], f32, tag="y")
                    nc.scalar.copy(out=y_t[:, 0:512], in_=psy[:, 0:512])
                    nc.vector.tensor_copy(
                        out=y_t[:, 512:1024], in_=psy[:, 512:1024])
                    nc.sync.dma_start(out=y[tsl, :], in_=y_t)

    nc.compile()
    return nc


def _host_prep(x, ln_g, ln_b, w_qkv, b_qkv, w_rel, w_out, b_out):
    """Per-core input dicts. LN affine is folded into w_qkv/b_qkv; the q-side
    softmax scale is folded into wq/bq; relative-position bias becomes the
    rank-64 (Qb, Kb) factor pair; LN normalization itself is folded into
    rank-1/2 correction terms (colsum/bias rows) applied on device."""
    f32 = np.float32
    scale = HD ** -0.5
    W = (w_qkv.astype(f32) * ln_g.astype(f32)[:, None]).astype(f32)
    b_eff = (b_qkv.astype(f32) + ln_b.astype(f32) @ w_qkv.astype(f32)).astype(f32)

    # sinusoid tables (float64 for accuracy)
    omg = np.exp(np.arange(0, HD, 2, dtype=np.float64)
                 * (-math.log(10000.0) / HD))          # [32]
    ang = omg[:, None] * np.arange(T, dtype=np.float64)[None, :]  # [32, T]
    S, C = np.sin(ang), np.cos(ang)
    Kb = np.empty((HD, T), np.float64)
    Kb[0::2], Kb[1::2] = C, S
    Kb = Kb.astype(f32)

    in_maps = []
    for c in range(NCORES):
        bi, hg = divmod(c, NCORES // B)
        hs = hg * HPC * HD  # 256-wide column block of this core's heads
        wqc = np.ascontiguousarray(W[:, hs:hs + 256] * scale)
        bqc = (b_eff[hs:hs + 256] * scale).astype(np.float64)
        wkc = np.ascontiguousarray(W[:, D + hs:D + hs + 256])
        wvc = np.zeros((D, 260), f32)
        bvc = np.zeros((260,), np.float64)
        for h in range(HPC):
            csl = slice(2 * D + hs + h * HD, 2 * D + hs + (h + 1) * HD)
            wvc[:, h * 65:h * 65 + 64] = W[:, csl]
            bvc[h * 65:h * 65 + 64] = b_eff[csl]
            bvc[h * 65 + 64] = 1.0
        # correction factors: row0 = -colsum(W), row1 = bias
        qcor = np.stack([-wqc.astype(np.float64).sum(0), bqc]).astype(f32)
        kcor = (-wkc.astype(np.float64).sum(0))[None, :].astype(f32)
        vcor = np.stack([-wvc.astype(np.float64).sum(0), bvc]).astype(f32)
        qbc = np.empty((HPC, HD, T), np.float64)
        for h in range(HPC):
            w2a = w_rel[0::2, hg * HPC + h].astype(np.float64)
            w2a1 = w_rel[1::2, hg * HPC + h].astype(np.float64)
            qbc[h, 0::2] = w2a[:, None] * S + w2a1[:, None] * C
            qbc[h, 1::2] = -w2a[:, None] * C + w2a1[:, None] * S
        xb = np.ascontiguousarray(x[bi].astype(f32))
        in_maps.append({
            "x": xb, "xt": np.ascontiguousarray(xb.T),
            "wq": wqc, "wk": wkc, "wv": wvc,
            "qcor": qcor, "kcor": kcor, "vcor": vcor,
            "qb": np.ascontiguousarray(qbc.astype(f32)),
            "kb": Kb, "wo": np.ascontiguousarray(w_out[hs:hs + 256, :].astype(f32)),
        })
    return in_maps


def kernel(x, ln_g, ln_b, w_qkv, b_qkv, w_rel, w_out, b_out):
    from concourse.bass_utils import run_bass_kernel_spmd

    x = np.asarray(x)
    nc = _build_nc()
    in_maps = _host_prep(x, np.asarray(ln_g), np.asarray(ln_b),
                         np.asarray(w_qkv), np.asarray(b_qkv),
                         np.asarray(w_rel), np.asarray(w_out),
                         np.asarray(b_out))
    res = run_bass_kernel_spmd(nc, in_maps, list(range(NCORES)))
    kernel._last_result = res
    cpb = NCORES // B
    y = np.empty((B, T, D), np.float32)
    for bi in range(B):
        acc = res.results[bi * cpb]["y"].astype(np.float32)
        for g in range(1, cpb):
            acc = acc + res.results[bi * cpb + g]["y"]
        y[bi] = acc + np.asarray(b_out, np.float32)[None, :]
    return y


# revision 12
# speedup vs baseline: 1.6205x; 1.6205x over previous
"""Trainium2 Bass kernel for MultiHeadSelfAttention with relative position bias.

Sharding: 8 cores = 2 batches x 4 head-groups (4 heads each).
Each core computes LN -> QKV (its heads) -> scores+softmax+AV -> partial
out_proj; host sums the 4 partials per batch (the all-reduce) and adds b_out.

Tricks:
1. rel_bias[i-j] is exactly rank-64 in (i, j) (sinusoid angle-addition), so
   it folds into the scores matmul as 64 extra contraction rows:
       scores+bias = [kT; Kb]^T @ [qT_scaled; Qb]   (K = 128, full PE array)
2. LayerNorm never materializes xn: projections run on unnormalized xT
   (host-transposed), and LN folds in algebraically:
       (xn @ W)^T[c,i] = rstd_i * (P[c,i] - mu_i * colsum(W)_c) + b_c
   The rank-1/2 corrections are tiny K<=2 matmuls accumulated into the same
   PSUM group; the rstd_i scaling rides the PSUM->SBUF evacuation.
3. Softmax denominators come free via an all-ones column appended to V
   (row 64 of the AV output = sum_j exp); normalization is deferred across
   the (linear) AV matmul into the OT evacuation.
"""

import functools
import math
import sys

import numpy as np

for _p in ("/opt/trn_rl_repo", "/root/.axon_site/_ro/trn_rl_repo"):
    if _p not in sys.path:
        sys.path.insert(0, _p)

B, T, D, H, HD = 2, 2048, 1024, 16, 64
NCORES = 8
HPC = 4  # heads per core
NT = T // 128  # 16 row tiles
NI = T // 512  # 4 i-chunks
F32R = True  # float32r (full-rate) matmuls; False = exact fp32 at 1/4 rate


@functools.lru_cache(maxsize=1)
def _build_nc():
    import concourse.mybir as mybir
    import concourse.tile as tile
    from concourse import bacc

    dt = mybir.dt
    f32 = dt.float32
    f32r = dt.float32r if F32R else f32
    AF = mybir.ActivationFunctionType

    nc = bacc.Bacc("TRN2", target_bir_lowering=False, debug=False,
                   num_devices=NCORES)
    xt = nc.declare_dram_parameter("xt", [D, T], f32r, isOutput=False)
    wq = nc.declare_dram_parameter("wq", [128, 8, 256], f32r, isOutput=False)
    wk = nc.declare_dram_parameter("wk", [128, 8, 256], f32r, isOutput=False)
    wv = nc.declare_dram_parameter("wv", [128, 8, 260], f32r, isOutput=False)
    onesc = nc.declare_dram_parameter("onesc", [128, 1], f32r, isOutput=False)
    idin = nc.declare_dram_parameter("ident", [1, 1], f32, isOutput=False)
    qcor = nc.declare_dram_parameter("qcor", [2, 256], f32, isOutput=False)
    kcor = nc.declare_dram_parameter("kcor", [1, 256], f32, isOutput=False)
    vcor = nc.declare_dram_parameter("vcor", [2, 260], f32, isOutput=False)
    qb = nc.declare_dram_parameter("qb", [HPC, 64, T], f32r, isOutput=False)
    kb = nc.declare_dram_parameter("kb", [64, T], f32r, isOutput=False)
    wo = nc.declare_dram_parameter("wo", [128, 2, D], f32r, isOutput=False)
    y = nc.declare_dram_parameter("y", [T, D], f32, isOutput=True)

    with tile.TileContext(nc) as tc:
        with tc.tile_pool(name="persist", bufs=1) as pp, \
             tc.tile_pool(name="big_ps", bufs=2, space="PSUM") as bigp, \
             tc.tile_pool(name="ot_ps", bufs=2, space="PSUM") as otpp:
            zero_col = pp.tile([128, 1], f32, tag="zero")
            nc.vector.memset(zero_col, 0.0)
            qhat = [pp.tile([128, T], f32r, tag=f"qhat{h}", name=f"qhat{h}")
                    for h in range(HPC)]
            khat = [pp.tile([128, T], f32r, tag=f"khat{h}", name=f"khat{h}")
                    for h in range(HPC)]
            v_sb = [pp.tile([128, 260], f32r, tag=f"v{jt}", name=f"v{jt}")
                    for jt in range(NT)]
            ot_pair = [pp.tile([128, T], f32r, tag=f"ot{p}", name=f"otp{p}")
                       for p in range(2)]
            wq_sb = pp.tile([128, 8, 256], f32r, tag="wq")
            wk_sb = pp.tile([128, 8, 256], f32r, tag="wk")
            wv_sb = pp.tile([128, 8, 260], f32r, tag="wv")

            # ------------- Phase A+B interleaved per i-group -------------
            # LN stats come from ones-column matmuls over xT (sum, sum of
            # squares per token); normalization folds into the projections.
            with tc.tile_pool(name="ab_sb", bufs=1) as abp, \
                 tc.tile_pool(name="rows", bufs=2) as rwp, \
                 tc.tile_pool(name="cols", bufs=1) as clp, \
                 tc.tile_pool(name="xts", bufs=2) as xtp, \
                 tc.tile_pool(name="x2s", bufs=1) as x2p:
                eps_t = abp.tile([1, 1], f32, tag="eps")
                nc.vector.memset(eps_t, 1e-5)
                c1024 = abp.tile([1, 1], f32, tag="c1024")
                nc.vector.memset(c1024, 1.0 / D)
                ident1 = abp.tile([1, 1], f32, tag="ident1")
                nc.sync.dma_start(out=ident1, in_=idin[:, :])
                ones_c = abp.tile([128, 1], f32r, tag="onesc")
                nc.sync.dma_start(out=ones_c, in_=onesc[:, :])
                qcs_sb = abp.tile([1, 256], f32, tag="qcs")
                nc.sync.dma_start(out=qcs_sb, in_=qcor[0:1, :])
                qbb_sb = abp.tile([1, 256], f32, tag="qbb")
                nc.sync.dma_start(out=qbb_sb, in_=qcor[1:2, :])
                kcor_sb = abp.tile([1, 256], f32, tag="kcor")
                nc.sync.dma_start(out=kcor_sb, in_=kcor[:, :])
                vcs_sb = abp.tile([1, 260], f32, tag="vcs")
                nc.sync.dma_start(out=vcs_sb, in_=vcor[0:1, :])
                vbb_sb = abp.tile([1, 260], f32, tag="vbb")
                nc.sync.dma_start(out=vbb_sb, in_=vcor[1:2, :])
                nc.sync.dma_start(out=wq_sb, in_=wq[:, :, :])
                nc.sync.dma_start(out=wk_sb, in_=wk[:, :, :])
                nc.sync.dma_start(out=wv_sb, in_=wv[:, :, :])

                for ig in range(NI):
                    isl = slice(ig * 512, (ig + 1) * 512)
                    xts = []
                    for kc in range(8):
                        xv = xtp.tile([128, 512], f32r, tag=f"x{kc}",
                                      name=f"xt{kc}_{ig}")
                        nc.sync.dma_start(
                            out=xv, in_=xt[kc * 128:(kc + 1) * 128, isl])
                        xts.append(xv)

                    # token sums / sums-of-squares via ones-column matmuls
                    ps_s = bigp.tile([128, 1024], f32, tag="big",
                                     name=f"stat{ig}")
                    x2s = []
                    for kc in range(8):
                        x2 = x2p.tile([128, 512], f32r, tag=f"x2{kc}",
                                      name=f"x2_{kc}_{ig}")
                        nc.vector.tensor_mul(out=x2, in0=xts[kc],
                                             in1=xts[kc])
                        x2s.append(x2)
                    for kc in range(8):
                        nc.tensor.matmul(
                            ps_s[0:1, 0:512], lhsT=ones_c, rhs=xts[kc],
                            start=(kc == 0), stop=(kc == 7))
                    for kc in range(8):
                        nc.tensor.matmul(
                            ps_s[0:1, 512:1024], lhsT=ones_c, rhs=x2s[kc],
                            start=(kc == 0), stop=(kc == 7))

                    # row math: mu, s=sqrt(var+eps), rstd=1/s
                    mu_r = rwp.tile([1, 512], f32, tag="mur",
                                    name=f"mur{ig}")
                    nc.vector.tensor_scalar_mul(
                        out=mu_r, in0=ps_s[0:1, 0:512], scalar1=c1024)
                    s_r = rwp.tile([1, 512], f32, tag="sr", name=f"sr{ig}")
                    # s_r <- sum(x^2)/D  (then -= mu^2, sqrt, +eps)
                    nc.vector.tensor_scalar_mul(
                        out=s_r, in0=ps_s[0:1, 512:1024], scalar1=c1024)
                    mu2 = rwp.tile([1, 512], f32, tag="mu2",
                                   name=f"mu2{ig}")
                    nc.vector.tensor_mul(out=mu2, in0=mu_r, in1=mu_r)
                    nc.vector.tensor_sub(out=s_r, in0=s_r, in1=mu2)
                    nc.scalar.activation(out=s_r, in_=s_r, func=AF.Sqrt,
                                         bias=eps_t, scale=1.0)
                    rst_r = rwp.tile([1, 512], f32, tag="rstr",
                                     name=f"rstr{ig}")
                    nc.vector.reciprocal_approx_fast(out=rst_r, in_=s_r)
                    rst_bc = rwp.tile([128, 512], f32, tag="rstb",
                                      name=f"rstb{ig}")
                    nc.gpsimd.partition_broadcast(rst_bc, rst_r)
                    # rstd as [128,1] columns for the v evacuation
                    rstd_cols = []
                    for r in range(4):
                        nc.tensor.transpose(
                            ps_s[:, 200 + r:201 + r],
                            rst_r[0:1, r * 128:(r + 1) * 128], ident1)
                        col = clp.tile([128, 1], f32, tag=f"rc{ig * 4 + r}",
                                       name=f"rc{ig * 4 + r}")
                        nc.vector.tensor_copy(
                            out=col, in_=ps_s[:, 200 + r:201 + r])
                        rstd_cols.append(col)

                    # q/k projections for this i-chunk (both cc in one tile)
                    for wt, dest, cors in ((wq_sb, qhat, (qcs_sb, qbb_sb)),
                                           (wk_sb, khat, (kcor_sb,))):
                        ps = bigp.tile([128, 1024], f32, tag="big",
                                       name=f"qk{ig}")
                        for cc in range(2):
                            psl = ps[:, cc * 512:(cc + 1) * 512]
                            csl = slice(cc * 128, (cc + 1) * 128)
                            for kc in range(8):
                                nc.tensor.matmul(
                                    psl, lhsT=wt[:, kc, csl], rhs=xts[kc],
                                    start=(kc == 0), stop=False)
                            nc.tensor.matmul(
                                psl, lhsT=cors[0][0:1, csl], rhs=mu_r,
                                start=False, stop=(len(cors) == 1))
                            if len(cors) == 2:
                                nc.tensor.matmul(
                                    psl, lhsT=cors[1][0:1, csl], rhs=s_r,
                                    start=False, stop=True)
                            nc.vector.tensor_mul(
                                out=dest[2 * cc][0:64, isl],
                                in0=psl[0:64, :], in1=rst_bc[0:64, :])
                            nc.vector.tensor_mul(
                                out=dest[2 * cc + 1][0:64, isl],
                                in0=psl[64:128, :], in1=rst_bc[64:128, :])

                    # v projections for the 4 j-tiles of this group
                    for jp in range(2):
                        ps = bigp.tile([128, 1024], f32, tag="big",
                                       name=f"vv{ig}_{jp}")
                        for half in range(2):
                            r = jp * 2 + half
                            jt = ig * 4 + r
                            rsl = slice(r * 128, (r + 1) * 128)
                            psl = ps[:, half * 512:half * 512 + 260]
                            for kc in range(8):
                                nc.tensor.matmul(
                                    psl, lhsT=xts[kc][:, rsl],
                                    rhs=wv_sb[:, kc, :],
                                    start=(kc == 0), stop=False)
                            nc.tensor.matmul(
                                psl, lhsT=mu_r[0:1, rsl],
                                rhs=vcs_sb, start=False, stop=False)
                            nc.tensor.matmul(
                                psl, lhsT=s_r[0:1, rsl],
                                rhs=vbb_sb, start=False, stop=True)
                            nc.vector.tensor_scalar_mul(
                                out=v_sb[jt], in0=psl,
                                scalar1=rstd_cols[r])

                # bias factor tables (needed first by phase C)
                for h in range(HPC):
                    nc.sync.dma_start(out=qhat[h][64:128, :], in_=qb[h])
                    nc.sync.dma_start(out=khat[h][64:128, :], in_=kb[:, :])

            # ---------------- Phase C: attention per head ----------------
            with tc.tile_pool(name="pt", bufs=4) as ptp, \
                 tc.tile_pool(name="rr", bufs=4) as rrp, \
                 tc.tile_pool(name="rb", bufs=4) as rbp:
                for h in range(HPC):
                    vsl = slice(h * 65, (h + 1) * 65)
                    for ic2 in range(2):
                        ps_o = [otpp.tile([128, 512], f32, tag=f"ot{_o}",
                                          name=f"ot{_o}_{h}_{ic2}")
                                for _o in range(2)]
                        for jt in range(NT):
                            jsl = slice(jt * 128, (jt + 1) * 128)
                            st = bigp.tile([128, 1024], f32, tag="big",
                                           name=f"sc{h}_{ic2}_{jt}")
                            pt = ptp.tile([128, 1024], f32r, tag="pt")
                            for half in range(2):
                                i0 = ic2 * 1024 + half * 512
                                nc.tensor.matmul(
                                    st[:, half * 512:(half + 1) * 512],
                                    lhsT=khat[h][:, jsl],
                                    rhs=qhat[h][:, i0:i0 + 512],
                                    start=True, stop=True)
                            nc.scalar.activation(
                                out=pt, in_=st, func=AF.Exp,
                                bias=zero_col, scale=1.0)
                            for half in range(2):
                                nc.tensor.matmul(
                                    ps_o[half][0:65, :],
                                    lhsT=v_sb[jt][:, vsl],
                                    rhs=pt[:, half * 512:(half + 1) * 512],
                                    start=(jt == 0), stop=(jt == NT - 1))
                        hp, pair = h % 2, h // 2
                        for half in range(2):
                            i0 = ic2 * 1024 + half * 512
                            rr = rrp.tile([1, 512], f32, tag="rr")
                            # approx-recip mis-reads PSUM; bounce via SBUF
                            nc.vector.tensor_copy(
                                out=rr, in_=ps_o[half][64:65, :])
                            nc.vector.reciprocal_approx_fast(out=rr, in_=rr)
                            rb = rbp.tile([64, 512], f32, tag="rb")
                            nc.gpsimd.partition_broadcast(rb, rr)
                            nc.vector.tensor_mul(
                                out=ot_pair[pair][hp * 64:(hp + 1) * 64,
                                                  i0:i0 + 512],
                                in0=ps_o[half][0:64, :], in1=rb)

            # ---------------- Phase D: out projection ----------------
            with tc.tile_pool(name="yp", bufs=3) as yp, \
                 tc.tile_pool(name="wop", bufs=1) as wop:
                wo_sb = wop.tile([128, 2, D], f32r, tag="wo")
                nc.sync.dma_start(out=wo_sb, in_=wo[:, :, :])
                for it in range(NT):
                    tsl = slice(it * 128, (it + 1) * 128)
                    psy = bigp.tile([128, D], f32, tag="big", name=f"y{it}")
                    for nh in range(2):
                        for p in range(2):
                            nc.tensor.matmul(
                                psy[:, nh * 512:(nh + 1) * 512],
                                lhsT=ot_pair[p][:, tsl],
                                rhs=wo_sb[:, p, nh * 512:(nh + 1) * 512],
                                start=(p == 0), stop=(p == 1))
                    y_t = yp.tile([128, D], f32, tag="y")
                    nc.scalar.copy(out=y_t[:, 0:512], in_=psy[:, 0:512])
                    nc.vector.tensor_copy(
                        out=y_t[:, 512:1024], in_=psy[:, 512:1024])
                    nc.sync.dma_start(out=y[tsl, :], in_=y_t)

    nc.compile()
    return nc


def _host_prep(x, ln_g, ln_b, w_qkv, b_qkv, w_rel, w_out, b_out):
    """Per-core input dicts. LN affine is folded into w_qkv/b_qkv; the q-side
    softmax scale is folded into wq/bq; relative-position bias becomes the
    rank-64 (Qb, Kb) factor pair; LN normalization itself is folded into
    rank-1/2 correction terms (colsum/bias rows) applied on device."""
    f32 = np.float32
    scale = HD ** -0.5
    W = (w_qkv.astype(f32) * ln_g.astype(f32)[:, None]).astype(f32)
    b_eff = (b_qkv.astype(f32) + ln_b.astype(f32) @ w_qkv.astype(f32)).astype(f32)

    # sinusoid tables (float64 for accuracy)
    omg = np.exp(np.arange(0, HD, 2, dtype=np.float64)
                 * (-math.log(10000.0) / HD))          # [32]
    ang = omg[:, None] * np.arange(T, dtype=np.float64)[None, :]  # [32, T]
    S, C = np.sin(ang), np.cos(ang)
    Kb = np.empty((HD, T), np.float64)
    Kb[0::2], Kb[1::2] = C, S
    Kb = Kb.astype(f32)

    in_maps = []
    for c in range(NCORES):
        bi, hg = divmod(c, NCORES // B)
        hs = hg * HPC * HD  # 256-wide column block of this core's heads
        wqc = np.ascontiguousarray(W[:, hs:hs + 256] * scale)
        bqc = (b_eff[hs:hs + 256] * scale).astype(np.float64)
        wkc = np.ascontiguousarray(W[:, D + hs:D + hs + 256])
        wvc = np.zeros((D, 260), f32)
        bvc = np.zeros((260,), np.float64)
        for h in range(HPC):
            csl = slice(2 * D + hs + h * HD, 2 * D + hs + (h + 1) * HD)
            wvc[:, h * 65:h * 65 + 64] = W[:, csl]
            bvc[h * 65:h * 65 + 64] = b_eff[csl]
            bvc[h * 65 + 64] = 1.0
        # correction factors: row0 = -colsum(W), row1 = bias
        qcor = np.stack([-wqc.astype(np.float64).sum(0), bqc]).astype(f32)
        kcor = (-wkc.astype(np.float64).sum(0))[None, :].astype(f32)
        vcor = np.stack([-wvc.astype(np.float64).sum(0), bvc]).astype(f32)
        qbc = np.empty((HPC, HD, T), np.float64)
        for h in range(HPC):
            w2a = w_rel[0::2, hg * HPC + h].astype(np.float64)
            w2a1 = w_rel[1::2, hg * HPC + h].astype(np.float64)
            qbc[h, 0::2] = w2a[:, None] * S + w2a1[:, None] * C
            qbc[h, 1::2] = -w2a[:, None] * C + w2a1[:, None] * S
        in_maps.append({
            "xt": np.ascontiguousarray(x[bi].astype(f32).T),
            "wq": np.ascontiguousarray(wqc.reshape(8, 128, 256).transpose(1, 0, 2)),
            "wk": np.ascontiguousarray(wkc.reshape(8, 128, 256).transpose(1, 0, 2)),
            "wv": np.ascontiguousarray(wvc.reshape(8, 128, 260).transpose(1, 0, 2)),
            "qcor": qcor, "kcor": kcor, "vcor": vcor,
            "qb": np.ascontiguousarray(qbc.astype(f32)),
            "kb": Kb,
            "wo": np.ascontiguousarray(
                w_out[hs:hs + 256, :].astype(f32).reshape(2, 128, D)
                .transpose(1, 0, 2)),
            "onesc": np.ones((128, 1), f32),
            "ident": np.ones((1, 1), f32),
        })
    return in_maps


def kernel(x, ln_g, ln_b, w_qkv, b_qkv, w_rel, w_out, b_out):
    from concourse.bass_utils import run_bass_kernel_spmd

    x = np.asarray(x)
    nc = _build_nc()
    in_maps = _host_prep(x, np.asarray(ln_g), np.asarray(ln_b),
                         np.asarray(w_qkv), np.asarray(b_qkv),
                         np.asarray(w_rel), np.asarray(w_out),
                         np.asarray(b_out))
    res = run_bass_kernel_spmd(nc, in_maps, list(range(NCORES)))
    kernel._last_result = res
    cpb = NCORES // B
    y = np.empty((B, T, D), np.float32)
    for bi in range(B):
        acc = res.results[bi * cpb]["y"].astype(np.float32)
        for g in range(1, cpb):
            acc = acc + res.results[bi * cpb + g]["y"]
        y[bi] = acc + np.asarray(b_out, np.float32)[None, :]
    return y


# revision 13
# speedup vs baseline: 1.6722x; 1.0319x over previous
"""Trainium2 Bass kernel for MultiHeadSelfAttention with relative position bias.

Sharding: 8 cores = 2 batches x 4 head-groups (4 heads each).
Each core computes LN -> QKV (its heads) -> scores+softmax+AV -> partial
out_proj; host sums the 4 partials per batch (the all-reduce) and adds b_out.

Tricks:
1. rel_bias[i-j] is exactly rank-64 in (i, j) (sinusoid angle-addition), so
   it folds into the scores matmul as 64 extra contraction rows:
       scores+bias = [kT; Kb]^T @ [qT_scaled; Qb]   (K = 128, full PE array)
2. LayerNorm never materializes xn: projections run on unnormalized xT
   (host-transposed), and LN folds in algebraically:
       (xn @ W)^T[c,i] = rstd_i * (P[c,i] - mu_i * colsum(W)_c) + b_c
   The rank-1/2 corrections are tiny K<=2 matmuls accumulated into the same
   PSUM group; the rstd_i scaling rides the PSUM->SBUF evacuation.
3. Softmax denominators come free via an all-ones column appended to V
   (row 64 of the AV output = sum_j exp); normalization is deferred across
   the (linear) AV matmul into the OT evacuation.
"""

import functools
import math
import sys

import numpy as np

for _p in ("/opt/trn_rl_repo", "/root/.axon_site/_ro/trn_rl_repo"):
    if _p not in sys.path:
        sys.path.insert(0, _p)

B, T, D, H, HD = 2, 2048, 1024, 16, 64
NCORES = 8
HPC = 4  # heads per core
NT = T // 128  # 16 row tiles
NI = T // 512  # 4 i-chunks
F32R = True  # float32r (full-rate) matmuls; False = exact fp32 at 1/4 rate


@functools.lru_cache(maxsize=1)
def _build_nc():
    import concourse.mybir as mybir
    import concourse.tile as tile
    from concourse import bacc

    dt = mybir.dt
    f32 = dt.float32
    f32r = dt.float32r if F32R else f32
    AF = mybir.ActivationFunctionType

    nc = bacc.Bacc("TRN2", target_bir_lowering=False, debug=False,
                   num_devices=NCORES)
    xt = nc.declare_dram_parameter("xt", [NI, 8, 128, 512], f32r, isOutput=False)
    wq = nc.declare_dram_parameter("wq", [128, 8, 256], f32r, isOutput=False)
    wk = nc.declare_dram_parameter("wk", [128, 8, 256], f32r, isOutput=False)
    wv = nc.declare_dram_parameter("wv", [128, 8, 260], f32r, isOutput=False)
    onesc = nc.declare_dram_parameter("onesc", [128, 1], f32r, isOutput=False)
    idin = nc.declare_dram_parameter("ident", [1, 1], f32, isOutput=False)
    qcor = nc.declare_dram_parameter("qcor", [2, 256], f32, isOutput=False)
    kcor = nc.declare_dram_parameter("kcor", [1, 256], f32, isOutput=False)
    vcor = nc.declare_dram_parameter("vcor", [2, 260], f32, isOutput=False)
    qb = nc.declare_dram_parameter("qb", [HPC, 64, T], f32r, isOutput=False)
    kb = nc.declare_dram_parameter("kb", [64, T], f32r, isOutput=False)
    wo = nc.declare_dram_parameter("wo", [128, 2, D], f32r, isOutput=False)
    y = nc.declare_dram_parameter("y", [T, D], f32, isOutput=True)

    with tile.TileContext(nc) as tc:
        with tc.tile_pool(name="persist", bufs=1) as pp, \
             tc.tile_pool(name="big_ps", bufs=2, space="PSUM") as bigp, \
             tc.tile_pool(name="ot_ps", bufs=2, space="PSUM") as otpp:
            zero_col = pp.tile([128, 1], f32, tag="zero")
            nc.vector.memset(zero_col, 0.0)
            qhat = [pp.tile([128, T], f32r, tag=f"qhat{h}", name=f"qhat{h}")
                    for h in range(HPC)]
            khat = [pp.tile([128, T], f32r, tag=f"khat{h}", name=f"khat{h}")
                    for h in range(HPC)]
            v_sb = [pp.tile([128, 260], f32r, tag=f"v{jt}", name=f"v{jt}")
                    for jt in range(NT)]
            ot_pair = [pp.tile([128, T], f32r, tag=f"ot{p}", name=f"otp{p}")
                       for p in range(2)]
            wq_sb = pp.tile([128, 8, 256], f32r, tag="wq")
            wk_sb = pp.tile([128, 8, 256], f32r, tag="wk")
            wv_sb = pp.tile([128, 8, 260], f32r, tag="wv")

            # ------------- Phase A+B interleaved per i-group -------------
            # LN stats come from ones-column matmuls over xT (sum, sum of
            # squares per token); normalization folds into the projections.
            with tc.tile_pool(name="ab_sb", bufs=1) as abp, \
                 tc.tile_pool(name="rows", bufs=2) as rwp, \
                 tc.tile_pool(name="cols", bufs=1) as clp, \
                 tc.tile_pool(name="xts", bufs=2) as xtp, \
                 tc.tile_pool(name="x2s", bufs=1) as x2p:
                eps_t = abp.tile([1, 1], f32, tag="eps")
                nc.vector.memset(eps_t, 1e-5)
                c1024 = abp.tile([1, 1], f32, tag="c1024")
                nc.vector.memset(c1024, 1.0 / D)
                ident1 = abp.tile([1, 1], f32, tag="ident1")
                nc.sync.dma_start(out=ident1, in_=idin[:, :])
                ones_c = abp.tile([128, 1], f32r, tag="onesc")
                nc.sync.dma_start(out=ones_c, in_=onesc[:, :])
                qcs_sb = abp.tile([1, 256], f32, tag="qcs")
                nc.sync.dma_start(out=qcs_sb, in_=qcor[0:1, :])
                qbb_sb = abp.tile([1, 256], f32, tag="qbb")
                nc.sync.dma_start(out=qbb_sb, in_=qcor[1:2, :])
                kcor_sb = abp.tile([1, 256], f32, tag="kcor")
                nc.sync.dma_start(out=kcor_sb, in_=kcor[:, :])
                vcs_sb = abp.tile([1, 260], f32, tag="vcs")
                nc.sync.dma_start(out=vcs_sb, in_=vcor[0:1, :])
                vbb_sb = abp.tile([1, 260], f32, tag="vbb")
                nc.sync.dma_start(out=vbb_sb, in_=vcor[1:2, :])
                nc.sync.dma_start(out=wq_sb, in_=wq[:, :, :])
                nc.sync.dma_start(out=wk_sb, in_=wk[:, :, :])
                nc.sync.dma_start(out=wv_sb, in_=wv[:, :, :])

                for ig in range(NI):
                    isl = slice(ig * 512, (ig + 1) * 512)
                    xts = []
                    for kc in range(8):
                        xv = xtp.tile([128, 512], f32r, tag=f"x{kc}",
                                      name=f"xt{kc}_{ig}")
                        nc.sync.dma_start(out=xv, in_=xt[ig, kc])
                        xts.append(xv)

                    # token sums / sums-of-squares via ones-column matmuls
                    ps_s = bigp.tile([128, 1024], f32, tag="big",
                                     name=f"stat{ig}")
                    x2s = []
                    for kc in range(8):
                        x2 = x2p.tile([128, 512], f32r, tag=f"x2{kc}",
                                      name=f"x2_{kc}_{ig}")
                        nc.vector.tensor_mul(out=x2, in0=xts[kc],
                                             in1=xts[kc])
                        x2s.append(x2)
                    for kc in range(8):
                        nc.tensor.matmul(
                            ps_s[0:1, 0:512], lhsT=ones_c, rhs=xts[kc],
                            start=(kc == 0), stop=(kc == 7))
                    for kc in range(8):
                        nc.tensor.matmul(
                            ps_s[0:1, 512:1024], lhsT=ones_c, rhs=x2s[kc],
                            start=(kc == 0), stop=(kc == 7))

                    # row math: mu, s=sqrt(var+eps), rstd=1/s
                    mu_r = rwp.tile([1, 512], f32, tag="mur",
                                    name=f"mur{ig}")
                    nc.vector.tensor_scalar_mul(
                        out=mu_r, in0=ps_s[0:1, 0:512], scalar1=c1024)
                    s_r = rwp.tile([1, 512], f32, tag="sr", name=f"sr{ig}")
                    # s_r <- sum(x^2)/D  (then -= mu^2, sqrt, +eps)
                    nc.vector.tensor_scalar_mul(
                        out=s_r, in0=ps_s[0:1, 512:1024], scalar1=c1024)
                    mu2 = rwp.tile([1, 512], f32, tag="mu2",
                                   name=f"mu2{ig}")
                    nc.vector.tensor_mul(out=mu2, in0=mu_r, in1=mu_r)
                    nc.vector.tensor_sub(out=s_r, in0=s_r, in1=mu2)
                    nc.scalar.activation(out=s_r, in_=s_r, func=AF.Sqrt,
                                         bias=eps_t, scale=1.0)
                    rst_r = rwp.tile([1, 512], f32, tag="rstr",
                                     name=f"rstr{ig}")
                    nc.vector.reciprocal_approx_fast(out=rst_r, in_=s_r)
                    rst_bc = rwp.tile([128, 512], f32, tag="rstb",
                                      name=f"rstb{ig}")
                    nc.gpsimd.partition_broadcast(rst_bc, rst_r)
                    # rstd as [128,1] columns for the v evacuation
                    rstd_cols = []
                    ps_c = otpp.tile([128, 512], f32, tag="ot0",
                                     name=f"colps{ig}")
                    for r in range(4):
                        nc.tensor.transpose(
                            ps_c[:, r:r + 1],
                            rst_r[0:1, r * 128:(r + 1) * 128], ident1)
                        col = clp.tile([128, 1], f32, tag=f"rc{ig * 4 + r}",
                                       name=f"rc{ig * 4 + r}")
                        nc.vector.tensor_copy(out=col, in_=ps_c[:, r:r + 1])
                        rstd_cols.append(col)

                    # q/k projections for this i-chunk (both cc in one tile)
                    for wt, dest, cors in ((wq_sb, qhat, (qcs_sb, qbb_sb)),
                                           (wk_sb, khat, (kcor_sb,))):
                        ps = bigp.tile([128, 1024], f32, tag="big",
                                       name=f"qk{ig}")
                        for cc in range(2):
                            psl = ps[:, cc * 512:(cc + 1) * 512]
                            csl = slice(cc * 128, (cc + 1) * 128)
                            for kc in range(8):
                                nc.tensor.matmul(
                                    psl, lhsT=wt[:, kc, csl], rhs=xts[kc],
                                    start=(kc == 0), stop=False)
                            nc.tensor.matmul(
                                psl, lhsT=cors[0][0:1, csl], rhs=mu_r,
                                start=False, stop=(len(cors) == 1))
                            if len(cors) == 2:
                                nc.tensor.matmul(
                                    psl, lhsT=cors[1][0:1, csl], rhs=s_r,
                                    start=False, stop=True)
                            nc.vector.tensor_mul(
                                out=dest[2 * cc][0:64, isl],
                                in0=psl[0:64, :], in1=rst_bc[0:64, :])
                            nc.vector.tensor_mul(
                                out=dest[2 * cc + 1][0:64, isl],
                                in0=psl[64:128, :], in1=rst_bc[64:128, :])

                    # v projections for the 4 j-tiles of this group
                    for jp in range(2):
                        ps = bigp.tile([128, 1024], f32, tag="big",
                                       name=f"vv{ig}_{jp}")
                        for half in range(2):
                            r = jp * 2 + half
                            jt = ig * 4 + r
                            rsl = slice(r * 128, (r + 1) * 128)
                            psl = ps[:, half * 512:half * 512 + 260]
                            for kc in range(8):
                                nc.tensor.matmul(
                                    psl, lhsT=xts[kc][:, rsl],
                                    rhs=wv_sb[:, kc, :],
                                    start=(kc == 0), stop=False)
                            nc.tensor.matmul(
                                psl, lhsT=mu_r[0:1, rsl],
                                rhs=vcs_sb, start=False, stop=False)
                            nc.tensor.matmul(
                                psl, lhsT=s_r[0:1, rsl],
                                rhs=vbb_sb, start=False, stop=True)
                            nc.vector.tensor_scalar_mul(
                                out=v_sb[jt], in0=psl,
                                scalar1=rstd_cols[r])

                # bias factor tables (needed first by phase C)
                for h in range(HPC):
                    nc.sync.dma_start(out=qhat[h][64:128, :], in_=qb[h])
                    nc.sync.dma_start(out=khat[h][64:128, :], in_=kb[:, :])

            # ---------------- Phase C: attention per head ----------------
            with tc.tile_pool(name="pt", bufs=4) as ptp, \
                 tc.tile_pool(name="rr", bufs=4) as rrp, \
                 tc.tile_pool(name="rb", bufs=4) as rbp:
                for h in range(HPC):
                    vsl = slice(h * 65, (h + 1) * 65)
                    for ic2 in range(2):
                        ps_o = [otpp.tile([128, 512], f32, tag=f"ot{_o}",
                                          name=f"ot{_o}_{h}_{ic2}")
                                for _o in range(2)]
                        for jt in range(NT):
                            jsl = slice(jt * 128, (jt + 1) * 128)
                            st = bigp.tile([128, 1024], f32, tag="big",
                                           name=f"sc{h}_{ic2}_{jt}")
                            pt = ptp.tile([128, 1024], f32r, tag="pt")
                            for half in range(2):
                                i0 = ic2 * 1024 + half * 512
                                nc.tensor.matmul(
                                    st[:, half * 512:(half + 1) * 512],
                                    lhsT=khat[h][:, jsl],
                                    rhs=qhat[h][:, i0:i0 + 512],
                                    start=True, stop=True)
                            nc.scalar.activation(
                                out=pt, in_=st, func=AF.Exp,
                                bias=zero_col, scale=1.0)
                            for half in range(2):
                                nc.tensor.matmul(
                                    ps_o[half][0:65, :],
                                    lhsT=v_sb[jt][:, vsl],
                                    rhs=pt[:, half * 512:(half + 1) * 512],
                                    start=(jt == 0), stop=(jt == NT - 1))
                        hp, pair = h % 2, h // 2
                        for half in range(2):
                            i0 = ic2 * 1024 + half * 512
                            rr = rrp.tile([1, 512], f32, tag="rr")
                            # approx-recip mis-reads PSUM; bounce via SBUF
                            nc.vector.tensor_copy(
                                out=rr, in_=ps_o[half][64:65, :])
                            nc.vector.reciprocal_approx_fast(out=rr, in_=rr)
                            rb = rbp.tile([64, 512], f32, tag="rb")
                            nc.gpsimd.partition_broadcast(rb, rr)
                            nc.vector.tensor_mul(
                                out=ot_pair[pair][hp * 64:(hp + 1) * 64,
                                                  i0:i0 + 512],
                                in0=ps_o[half][0:64, :], in1=rb)

            # ---------------- Phase D: out projection ----------------
            with tc.tile_pool(name="yp", bufs=3) as yp, \
                 tc.tile_pool(name="wop", bufs=1) as wop:
                wo_sb = wop.tile([128, 2, D], f32r, tag="wo")
                nc.sync.dma_start(out=wo_sb, in_=wo[:, :, :])
                for it in range(NT):
                    tsl = slice(it * 128, (it + 1) * 128)
                    psy = bigp.tile([128, D], f32, tag="big", name=f"y{it}")
                    for nh in range(2):
                        for p in range(2):
                            nc.tensor.matmul(
                                psy[:, nh * 512:(nh + 1) * 512],
                                lhsT=ot_pair[p][:, tsl],
                                rhs=wo_sb[:, p, nh * 512:(nh + 1) * 512],
                                start=(p == 0), stop=(p == 1))
                    y_t = yp.tile([128, D], f32, tag="y")
                    nc.scalar.copy(out=y_t[:, 0:512], in_=psy[:, 0:512])
                    nc.vector.tensor_copy(
                        out=y_t[:, 512:1024], in_=psy[:, 512:1024])
                    nc.sync.dma_start(out=y[tsl, :], in_=y_t)

    nc.compile()
    return nc


def _host_prep(x, ln_g, ln_b, w_qkv, b_qkv, w_rel, w_out, b_out):
    """Per-core input dicts. LN affine is folded into w_qkv/b_qkv; the q-side
    softmax scale is folded into wq/bq; relative-position bias becomes the
    rank-64 (Qb, Kb) factor pair; LN normalization itself is folded into
    rank-1/2 correction terms (colsum/bias rows) applied on device."""
    f32 = np.float32
    scale = HD ** -0.5
    W = (w_qkv.astype(f32) * ln_g.astype(f32)[:, None]).astype(f32)
    b_eff = (b_qkv.astype(f32) + ln_b.astype(f32) @ w_qkv.astype(f32)).astype(f32)

    # sinusoid tables (float64 for accuracy)
    omg = np.exp(np.arange(0, HD, 2, dtype=np.float64)
                 * (-math.log(10000.0) / HD))          # [32]
    ang = omg[:, None] * np.arange(T, dtype=np.float64)[None, :]  # [32, T]
    S, C = np.sin(ang), np.cos(ang)
    Kb = np.empty((HD, T), np.float64)
    Kb[0::2], Kb[1::2] = C, S
    Kb = Kb.astype(f32)

    in_maps = []
    for c in range(NCORES):
        bi, hg = divmod(c, NCORES // B)
        hs = hg * HPC * HD  # 256-wide column block of this core's heads
        wqc = np.ascontiguousarray(W[:, hs:hs + 256] * scale)
        bqc = (b_eff[hs:hs + 256] * scale).astype(np.float64)
        wkc = np.ascontiguousarray(W[:, D + hs:D + hs + 256])
        wvc = np.zeros((D, 260), f32)
        bvc = np.zeros((260,), np.float64)
        for h in range(HPC):
            csl = slice(2 * D + hs + h * HD, 2 * D + hs + (h + 1) * HD)
            wvc[:, h * 65:h * 65 + 64] = W[:, csl]
            bvc[h * 65:h * 65 + 64] = b_eff[csl]
            bvc[h * 65 + 64] = 1.0
        # correction factors: row0 = -colsum(W), row1 = bias
        qcor = np.stack([-wqc.astype(np.float64).sum(0), bqc]).astype(f32)
        kcor = (-wkc.astype(np.float64).sum(0))[None, :].astype(f32)
        vcor = np.stack([-wvc.astype(np.float64).sum(0), bvc]).astype(f32)
        qbc = np.empty((HPC, HD, T), np.float64)
        for h in range(HPC):
            w2a = w_rel[0::2, hg * HPC + h].astype(np.float64)
            w2a1 = w_rel[1::2, hg * HPC + h].astype(np.float64)
            qbc[h, 0::2] = w2a[:, None] * S + w2a1[:, None] * C
            qbc[h, 1::2] = -w2a[:, None] * C + w2a1[:, None] * S
        in_maps.append({
            "xt": np.ascontiguousarray(
                x[bi].astype(f32).T.reshape(8, 128, NI, 512)
                .transpose(2, 0, 1, 3)),
            "wq": np.ascontiguousarray(wqc.reshape(8, 128, 256).transpose(1, 0, 2)),
            "wk": np.ascontiguousarray(wkc.reshape(8, 128, 256).transpose(1, 0, 2)),
            "wv": np.ascontiguousarray(wvc.reshape(8, 128, 260).transpose(1, 0, 2)),
            "qcor": qcor, "kcor": kcor, "vcor": vcor,
            "qb": np.ascontiguousarray(qbc.astype(f32)),
            "kb": Kb,
            "wo": np.ascontiguousarray(
                w_out[hs:hs + 256, :].astype(f32).reshape(2, 128, D)
                .transpose(1, 0, 2)),
            "onesc": np.ones((128, 1), f32),
            "ident": np.ones((1, 1), f32),
        })
    return in_maps


def kernel(x, ln_g, ln_b, w_qkv, b_qkv, w_rel, w_out, b_out):
    from concourse.bass_utils import run_bass_kernel_spmd

    x = np.asarray(x)
    nc = _build_nc()
    in_maps = _host_prep(x, np.asarray(ln_g), np.asarray(ln_b),
                         np.asarray(w_qkv), np.asarray(b_qkv),
                         np.asarray(w_rel), np.asarray(w_out),
                         np.asarray(b_out))
    res = run_bass_kernel_spmd(nc, in_maps, list(range(NCORES)))
    kernel._last_result = res
    cpb = NCORES // B
    y = np.empty((B, T, D), np.float32)
    for bi in range(B):
        acc = res.results[bi * cpb]["y"].astype(np.float32)
        for g in range(1, cpb):
            acc = acc + res.results[bi * cpb + g]["y"]
        y[bi] = acc + np.asarray(b_out, np.float32)[None, :]
    return y
